# revision 26
# baseline (speedup 1.0000x reference)
"""GRU cell on 8 Trainium2 NeuronCores — data-parallel over batch.

Math (per batch row):
    z = sigmoid([x, h] @ W_z + b_z)
    r = sigmoid([x, h] @ W_r + b_r)
    n = tanh(x @ W_n[:D] + (r * h) @ W_n[D:] + b_n)
    h' = (1 - z) * h + z * n = h + z * (n - h)

Distribution: batch 8192 is split 1024 rows per core; weights are
replicated. Everything on-device is computed in a transposed layout
[hidden, batch] so both matmul operands have the contraction dim on
SBUF partitions and no on-device transpose is needed:
    out.T[ho, b] = sum_k W[k, ho] * xh.T[k, b]
The host pre-transposes x/h (free) and transposes the result back.

Matmuls run in fp8 e4m3 with perf_mode=DoubleRow (two k-rows packed
per PE cell -> 2 MACs/cell/cycle, fp32 PSUM accumulation). Weights
are pre-scaled by 128 on the host so their [-1/64, 1/64] range uses
the e4m3 normal range; the 1/128 is folded into the activation's
scale operand. h_prev is kept in bf16 for the elementwise combine so
fp8 error only enters through the matmuls.

Weights and activations are pre-swizzled on the host into the exact
SBUF layout ([partition, tile, ...]) so every DMA is a long
contiguous line per partition.
"""

import os
import sys
import types
from types import SimpleNamespace

import numpy as np

import concourse.bass as bass
import concourse.tile as tile
from concourse import bacc, mybir
from concourse._compat import with_exitstack
from concourse.bass_interp import get_hw_module
from concourse.bass_utils import run_bass_kernel_spmd

try:
    from ml_dtypes import bfloat16 as np_bf16
    from ml_dtypes import float8_e4m3 as np_f8
except ImportError:  # pragma: no cover
    import jax.numpy as jnp

    np_bf16 = jnp.bfloat16
    np_f8 = jnp.float8_e4m3

f32 = mybir.dt.float32
bf16 = mybir.dt.bfloat16
fp8 = mybir.dt.float8e4
DoubleRow = mybir.MatmulPerfMode.DoubleRow

N_CORES = 8
P = 128  # partitions
WSCALE = 128.0  # host weight pre-scale; 1/WSCALE folded into activation


def _default_cfg():
    D = 2048  # input size
    H = 2048  # hidden size
    return SimpleNamespace(
        D=D,
        H=H,
        BATCH=8192,
        BC=8192 // N_CORES,  # batch per core (1024)
        K=D + H,  # contraction dim (4096)
        KT=(D + H) // P,  # k-chunks (32)
        DT=D // P,  # k-chunks covering the x part (16)
        HT=H // P,  # k-chunks covering the h part (16)
        JT=H // P,  # hidden-out tiles (16)
        NF=512,  # moving free dim per matmul (one PSUM bank of fp32)
    )


CFG = _default_cfg()


def _install_ntff_hook():
    """antenv.axon_hooks isn't injected in this image; shim it so
    run_bass_kernel_spmd(trace=True) can capture NTFF profiles."""
    if "antenv.axon_hooks" in sys.modules:
        return
    try:
        from trn_agent_boot.trn_boot import _ntff_profile_via_ctypes

        hook = _ntff_profile_via_ctypes("/opt/axon/libaxon_pjrt.so")
    except Exception:
        hook = None
    mod = types.ModuleType("antenv.axon_hooks")
    mod.get_axon_ntff_profile_hook = lambda: hook
    mod.set_axon_ntff_profile_hook = lambda h: None
    sys.modules["antenv.axon_hooks"] = mod


@with_exitstack
def _gru_tile_kernel(ctx, tc, cfg, xh8, hb, wz, wr, wn, bz, br, bn, out):
    nc = tc.nc
    Sigmoid = mybir.ActivationFunctionType.Sigmoid
    Tanh = mybir.ActivationFunctionType.Tanh
    BC, KT, DT, HT, JT, NF = cfg.BC, cfg.KT, cfg.DT, cfg.HT, cfg.JT, cfg.NF
    NB = BC // NF  # batch blocks per core
    TP = KT // 2  # DoubleRow k-pairs
    XP = DT // 2  # k-pairs covering the x part
    INV = 1.0 / WSCALE

    const_pool = ctx.enter_context(tc.tile_pool(name="const", bufs=1))
    xh_pool = ctx.enter_context(tc.tile_pool(name="xhp", bufs=1))
    h_pool = ctx.enter_context(tc.tile_pool(name="hp", bufs=1))
    rh_pool = ctx.enter_context(tc.tile_pool(name="rhp", bufs=1))
    w_pool = ctx.enter_context(tc.tile_pool(name="wp", bufs=5))
    act_pool = ctx.enter_context(tc.tile_pool(name="actp", bufs=3))
    out_pool = ctx.enter_context(tc.tile_pool(name="outp", bufs=3))
    psum_pool = ctx.enter_context(tc.tile_pool(name="psp", bufs=8, space="PSUM"))

    # Input DMAs go on the Activation engine's DMA ring so they stream in
    # parallel with the weight tiles on the Sync ring (16 shared HW engines
    # service both); otherwise the first weight tile queues behind ~8 MB of
    # activations and the PE idles ~40 us before the first matmul.

    # PE warmup: the HAM clock gate holds the PE at 1.2 GHz until it has
    # seen ~3.4 us of sustained matmul activity; bridge most of the gap to
    # the first data landing (~11 us) with dummy matmuls on memset scratch.
    warm_sb = const_pool.tile([P, 2, 256], fp8, name="warm_sb")
    nc.vector.memset(warm_sb[:], 0.0)
    warm_ps = psum_pool.tile([P, 256], f32, tag="ps", name="warm_ps")
    for _ in range(8):
        nc.tensor.matmul(
            warm_ps[:], warm_sb[:, :, :128], warm_sb[:], perf_mode=DoubleRow
        )

    # Resident activations: xh.T as [128, KT, BC] fp8 (pre-swizzled in DRAM,
    # so each DMA is one contiguous line per partition). Quartered so the
    # first k-pairs land (and matmuls start) before the whole 4 MB arrives.
    # xh is alone on the Activation engine's DMA ring.
    xh_sb = xh_pool.tile([P, KT, BC], fp8, name="xh_sb")
    XQ = max(KT // 4, 1)
    for q in range(KT // XQ):
        nc.scalar.dma_start(
            xh_sb[:, q * XQ : (q + 1) * XQ, :], xh8[:, q * XQ : (q + 1) * XQ, :]
        )

    # Biases as [128, JT] (pre-swizzled on host: column j holds
    # bias[j*128:(j+1)*128]) so the DMA is one contiguous line per partition.
    bias_sb = {}
    for name, ap in (("z", bz), ("r", br), ("n", bn)):
        t = const_pool.tile([P, JT], f32, name=f"bias_{name}")
        nc.scalar.dma_start(t[:], ap)
        bias_sb[name] = t

    def load_w(w_ap, j, name):
        """[128, KT, 128] fp8 tile: [:, t, :] holds W[t*128+p, j*128+..]."""
        wt = w_pool.tile([P, KT, P], fp8, tag="w", name=name)
        nc.sync.dma_start(wt[:], w_ap[:, j : j + 1, :, :])
        return wt

    # First JB+1 r-gate weight tiles prefetch on the sync ring up front.
    # Only wr_0 transfers immediately; wr_1.. are gated behind the warmup
    # (a 1-element junk write each tile's DMA must wait on) so the startup
    # DMA window carries just wr_0 + xh q0 — the minimum for the first
    # matmuls — and the rest arrives just-in-time for its slot in the
    # quarter-group schedule below.
    JB = 4 if JT % 4 == 0 else 1  # leading pair-outer j-block width
    wr_pre = {}
    for j in range(min(JB + 1, JT)):
        wt = w_pool.tile([P, KT, P], fp8, tag="w", name="wr_j")
        if j > 0:
            nc.vector.tensor_copy(wt[:, 0:1, 0:1], warm_ps[:, 0:1])
        nc.sync.dma_start(wt[:], wr[:, j : j + 1, :, :])
        wr_pre[j] = wt

    # h_prev.T in bf16 for the elementwise tail (fp8 h would dominate error).
    h_sb = h_pool.tile([P, HT, BC], bf16, name="h_sb")

    # r * h_prev (transposed) in fp8, filled during the r phase.
    rh_sb = rh_pool.tile([P, HT, BC], fp8, name="rh_sb")

    def xh_pair(tp, b_i):
        return xh_sb[:, 2 * tp : 2 * tp + 2, b_i * NF : (b_i + 1) * NF]

    def accumulate(ps, w_tile, rhs_of_pair):
        for tp in range(TP):
            lhsT = w_tile[:, 2 * tp : 2 * tp + 2, :]
            for b_i in range(NB):
                nc.tensor.matmul(
                    ps[b_i][:],
                    lhsT,
                    rhs_of_pair(tp, b_i),
                    start=(tp == 0),
                    stop=(tp == TP - 1),
                    perf_mode=DoubleRow,
                )

    def r_tail(j, ps):
        r_j = act_pool.tile([P, BC], bf16, tag="r", name="r_j")
        for b_i in range(NB):
            nc.scalar.activation(
                r_j[:, b_i * NF : (b_i + 1) * NF],
                ps[b_i][:],
                Sigmoid,
                bias=bias_sb["r"][:, j : j + 1],
                scale=INV,
            )
        nc.vector.tensor_mul(rh_sb[:, j : j + 1, :], r_j[:], h_sb[:, j : j + 1, :])

    # ---- phase R: r gate, then rh = r * h_prev ----
    # The first JB tiles accumulate pair-outer in lockstep (JB*NB PSUM
    # groups): the PE consumes each freshly-DMA'd xh quarter JB x slower
    # than a single tile would, so the initial xh stream hides completely
    # under matmuls instead of stalling tile 0. (The HAM clock also warms
    # up during this stretch.)
    # Pair quarter-groups aligned with the xh quarter DMAs; within a
    # quarter go j-sequential so the first matmuls need only wr_0 + xh q0
    # and each later tile/quarter arrives just-in-time.
    pss = {j: [psum_pool.tile([P, NF], f32, tag="ps", name="ps_r")
               for _ in range(NB)] for j in range(JB)}
    TQ = max(TP // 4, 1)
    for tq in range(TP // TQ):
        for j in range(JB):
            for tp in range(tq * TQ, (tq + 1) * TQ):
                lhsT = wr_pre[j][:, 2 * tp : 2 * tp + 2, :]
                for b_i in range(NB):
                    nc.tensor.matmul(
                        pss[j][b_i][:],
                        lhsT,
                        xh_pair(tp, b_i),
                        start=(tp == 0),
                        stop=(tp == TP - 1),
                        perf_mode=DoubleRow,
                    )

    # Gate h's 4 MB DMA behind the first matmul block: a 1-element copy
    # from a block PSUM into h_sb makes the (whole-tile) h DMA a
    # write-after-write successor, so its transfer doesn't steal startup
    # bandwidth from xh/weights. The DMA then overwrites the junk element
    # with the real value. h is only consumed by rh and the combine, both
    # far later.
    nc.vector.tensor_copy(h_sb[:, 0:1, 0:1], pss[0][0][:, 0:1])
    nc.sync.dma_start(h_sb[:, : HT // 2, :], hb[:, : HT // 2, :])
    nc.sync.dma_start(h_sb[:, HT // 2 :, :], hb[:, HT // 2 :, :])

    for j in range(JB):
        r_tail(j, pss[j])

    for j in range(JB, JT):
        wr_j = wr_pre.pop(j) if j in wr_pre else load_w(wr, j, "wr_j")
        ps = [psum_pool.tile([P, NF], f32, tag="ps", name="ps_r") for _ in range(NB)]
        accumulate(ps, wr_j, xh_pair)
        r_tail(j, ps)

    # ---- phase NZ: z and n gates + combine ----
    for j in range(JT):
        wn_j = load_w(wn, j, "wn_j")
        wz_j = load_w(wz, j, "wz_j")
        psn = [psum_pool.tile([P, NF], f32, tag="ps", name="ps_n") for _ in range(NB)]
        psz = [psum_pool.tile([P, NF], f32, tag="ps", name="ps_z") for _ in range(NB)]

        def n_rhs(tp, b_i):
            if tp < XP:
                return xh_pair(tp, b_i)
            tt = tp - XP
            return rh_sb[:, 2 * tt : 2 * tt + 2, b_i * NF : (b_i + 1) * NF]

        # n-gate matmuls first: its activation + (n - h) then overlap the
        # z-gate matmuls, leaving only sigmoid -> mul -> add -> store after
        # the last matmul. Intermediates in bf16 for 2x DVE throughput.
        accumulate(psn, wn_j, n_rhs)
        n_j = act_pool.tile([P, BC], bf16, tag="n", name="n_j")
        d_j = act_pool.tile([P, BC], bf16, tag="d", name="d_j")
        for b_i in range(NB):
            sl = slice(b_i * NF, (b_i + 1) * NF)
            nc.scalar.activation(
                n_j[:, sl], psn[b_i][:], Tanh,
                bias=bias_sb["n"][:, j : j + 1], scale=INV,
            )
            nc.vector.tensor_sub(d_j[:, sl], n_j[:, sl], h_sb[:, j : j + 1, sl])

        accumulate(psz, wz_j, xh_pair)
        z_j = act_pool.tile([P, BC], bf16, tag="z", name="z_j")
        zd_j = act_pool.tile([P, BC], bf16, tag="zd", name="zd_j")
        o_j = out_pool.tile([P, BC], f32, name="o_j")
        # Last tile: quarter the post-matmul chain so the final
        # sigmoid->mul->add->store tail is 4x shorter.
        QS = NF // 2 if j == JT - 1 else NF
        for q in range(BC // QS):
            sl = slice(q * QS, (q + 1) * QS)
            h_b = h_sb[:, j : j + 1, sl]
            ps_q = psz[q * QS // NF][:, q * QS % NF : q * QS % NF + QS]
            nc.scalar.activation(
                z_j[:, sl], ps_q, Sigmoid,
                bias=bias_sb["z"][:, j : j + 1], scale=INV,
            )
            nc.vector.tensor_mul(zd_j[:, sl], z_j[:, sl], d_j[:, sl])
            nc.vector.tensor_add(o_j[:, sl], zd_j[:, sl], h_b)
            nc.sync.dma_start(out[j * P : (j + 1) * P, sl], o_j[:, sl])


_CACHED = {}


def _build(cfg=CFG, key="full", hw=True):
    if key in _CACHED:
        return _CACHED[key]
    nc = bacc.Bacc(
        "TRN2", target_bir_lowering=False, debug=False, enable_asserts=False
    )
    BC, K, H, KT, JT = cfg.BC, cfg.K, cfg.H, cfg.KT, cfg.JT
    xh8 = nc.dram_tensor("xh8", [P, KT, BC], fp8, kind="ExternalInput").ap()
    hb = nc.dram_tensor("hb", [P, cfg.HT, BC], bf16, kind="ExternalInput").ap()
    wz = nc.dram_tensor("wz", [P, JT, KT, P], fp8, kind="ExternalInput").ap()
    wr = nc.dram_tensor("wr", [P, JT, KT, P], fp8, kind="ExternalInput").ap()
    wn = nc.dram_tensor("wn", [P, JT, KT, P], fp8, kind="ExternalInput").ap()
    bz = nc.dram_tensor("bz", [P, JT], f32, kind="ExternalInput").ap()
    br = nc.dram_tensor("br", [P, JT], f32, kind="ExternalInput").ap()
    bn = nc.dram_tensor("bn", [P, JT], f32, kind="ExternalInput").ap()
    out = nc.dram_tensor("out", [H, BC], f32, kind="ExternalOutput").ap()

    with tile.TileContext(nc) as tc:
        _gru_tile_kernel(tc, cfg, xh8, hb, wz, wr, wn, bz, br, bn, out)
    nc.compile()
    if hw:
        nc.m = get_hw_module(nc.m)
    _CACHED[key] = nc
    return nc


def _swizzle_w(W, cfg):
    """[K, H] -> [P, JT, KT, P]: w[p, j, t, m] = W[t*128+p, j*128+m]."""
    w8 = (np.asarray(W, dtype=np.float32) * WSCALE).astype(np_f8)
    w8 = w8.reshape(cfg.KT, P, cfg.JT, P).transpose(1, 2, 0, 3)
    return np.ascontiguousarray(w8)


def _swizzle_act(a_t, nt, bc, dtype):
    """[nt*P, BC] -> [P, nt, BC]: out[p, t, :] = a_t[t*128+p, :]."""
    a = np.asarray(a_t).reshape(nt, P, bc).transpose(1, 0, 2)
    return np.ascontiguousarray(a.astype(dtype))


def _make_in_maps(x, h_prev, W_z, b_z, W_r, b_r, W_n, b_n, cfg=CFG):
    wz8 = _swizzle_w(W_z, cfg)
    wr8 = _swizzle_w(W_r, cfg)
    wn8 = _swizzle_w(W_n, cfg)

    def _swz_b(b):  # [H] -> [P, JT]: col j = b[j*128:(j+1)*128]
        return np.ascontiguousarray(
            np.asarray(b, dtype=np.float32).reshape(cfg.JT, P).T
        )

    bz32 = _swz_b(b_z)
    br32 = _swz_b(b_r)
    bn32 = _swz_b(b_n)
    in_maps = []
    for i in range(N_CORES):
        sl = slice(i * cfg.BC, (i + 1) * cfg.BC)
        xh_i = np.concatenate([x[sl].T, h_prev[sl].T], axis=0)
        in_maps.append(
            {
                "xh8": _swizzle_act(xh_i, cfg.KT, cfg.BC, np_f8),
                "hb": _swizzle_act(h_prev[sl].T, cfg.HT, cfg.BC, np_bf16),
                "wz": wz8,
                "wr": wr8,
                "wn": wn8,
                "bz": bz32,
                "br": br32,
                "bn": bn32,
            }
        )
    return in_maps


LAST_RESULT = None


def kernel(x, h_prev, W_z, b_z, W_r, b_r, W_n, b_n):
    global LAST_RESULT
    trace = bool(os.environ.get("GRU_TRACE"))
    if trace:
        _install_ntff_hook()
    nc = _build()
    in_maps = _make_in_maps(x, h_prev, W_z, b_z, W_r, b_r, W_n, b_n)
    res = run_bass_kernel_spmd(
        nc, in_maps, core_ids=list(range(N_CORES)), trace=trace
    )
    LAST_RESULT = res
    outs = [res.results[i]["out"].T for i in range(N_CORES)]
    return np.ascontiguousarray(np.concatenate(outs, axis=0).astype(np.float32))


# revision 27
# speedup vs baseline: 1.0144x; 1.0144x over previous
"""GRU cell on 8 Trainium2 NeuronCores — data-parallel over batch.

Math (per batch row):
    z = sigmoid([x, h] @ W_z + b_z)
    r = sigmoid([x, h] @ W_r + b_r)
    n = tanh(x @ W_n[:D] + (r * h) @ W_n[D:] + b_n)
    h' = (1 - z) * h + z * n = h + z * (n - h)

Distribution: batch 8192 is split 1024 rows per core; weights are
replicated. Everything on-device is computed in a transposed layout
[hidden, batch] so both matmul operands have the contraction dim on
SBUF partitions and no on-device transpose is needed:
    out.T[ho, b] = sum_k W[k, ho] * xh.T[k, b]
The host pre-transposes x/h (free) and transposes the result back.

Matmuls run in fp8 e4m3 with perf_mode=DoubleRow (two k-rows packed
per PE cell -> 2 MACs/cell/cycle, fp32 PSUM accumulation). Weights
are pre-scaled by 128 on the host so their [-1/64, 1/64] range uses
the e4m3 normal range; the 1/128 is folded into the activation's
scale operand. h_prev is kept in bf16 for the elementwise combine so
fp8 error only enters through the matmuls.

Weights and activations are pre-swizzled on the host into the exact
SBUF layout ([partition, tile, ...]) so every DMA is a long
contiguous line per partition.
"""

import os
import sys
import types
from types import SimpleNamespace

import numpy as np

import concourse.bass as bass
import concourse.tile as tile
from concourse import bacc, mybir
from concourse._compat import with_exitstack
from concourse.bass_interp import get_hw_module
from concourse.bass_utils import run_bass_kernel_spmd

try:
    from ml_dtypes import bfloat16 as np_bf16
    from ml_dtypes import float8_e4m3 as np_f8
except ImportError:  # pragma: no cover
    import jax.numpy as jnp

    np_bf16 = jnp.bfloat16
    np_f8 = jnp.float8_e4m3

f32 = mybir.dt.float32
bf16 = mybir.dt.bfloat16
fp8 = mybir.dt.float8e4
DoubleRow = mybir.MatmulPerfMode.DoubleRow

N_CORES = 8
P = 128  # partitions
WSCALE = 128.0  # host weight pre-scale; 1/WSCALE folded into activation


def _default_cfg():
    D = 2048  # input size
    H = 2048  # hidden size
    return SimpleNamespace(
        D=D,
        H=H,
        BATCH=8192,
        BC=8192 // N_CORES,  # batch per core (1024)
        K=D + H,  # contraction dim (4096)
        KT=(D + H) // P,  # k-chunks (32)
        DT=D // P,  # k-chunks covering the x part (16)
        HT=H // P,  # k-chunks covering the h part (16)
        JT=H // P,  # hidden-out tiles (16)
        NF=512,  # moving free dim per matmul (one PSUM bank of fp32)
    )


CFG = _default_cfg()


def _install_ntff_hook():
    """antenv.axon_hooks isn't injected in this image; shim it so
    run_bass_kernel_spmd(trace=True) can capture NTFF profiles."""
    if "antenv.axon_hooks" in sys.modules:
        return
    try:
        from trn_agent_boot.trn_boot import _ntff_profile_via_ctypes

        hook = _ntff_profile_via_ctypes("/opt/axon/libaxon_pjrt.so")
    except Exception:
        hook = None
    mod = types.ModuleType("antenv.axon_hooks")
    mod.get_axon_ntff_profile_hook = lambda: hook
    mod.set_axon_ntff_profile_hook = lambda h: None
    sys.modules["antenv.axon_hooks"] = mod


@with_exitstack
def _gru_tile_kernel(ctx, tc, cfg, xh8, hb, wz, wr, wn, bz, br, bn, out):
    nc = tc.nc
    Sigmoid = mybir.ActivationFunctionType.Sigmoid
    Tanh = mybir.ActivationFunctionType.Tanh
    BC, KT, DT, HT, JT, NF = cfg.BC, cfg.KT, cfg.DT, cfg.HT, cfg.JT, cfg.NF
    NB = BC // NF  # batch blocks per core
    TP = KT // 2  # DoubleRow k-pairs
    XP = DT // 2  # k-pairs covering the x part
    INV = 1.0 / WSCALE

    const_pool = ctx.enter_context(tc.tile_pool(name="const", bufs=1))
    xh_pool = ctx.enter_context(tc.tile_pool(name="xhp", bufs=1))
    h_pool = ctx.enter_context(tc.tile_pool(name="hp", bufs=1))
    rh_pool = ctx.enter_context(tc.tile_pool(name="rhp", bufs=1))
    w_pool = ctx.enter_context(tc.tile_pool(name="wp", bufs=5))
    act_pool = ctx.enter_context(tc.tile_pool(name="actp", bufs=3))
    out_pool = ctx.enter_context(tc.tile_pool(name="outp", bufs=3))
    psum_pool = ctx.enter_context(tc.tile_pool(name="psp", bufs=8, space="PSUM"))

    # Input DMAs go on the Activation engine's DMA ring so they stream in
    # parallel with the weight tiles on the Sync ring (16 shared HW engines
    # service both); otherwise the first weight tile queues behind ~8 MB of
    # activations and the PE idles ~40 us before the first matmul.

    # PE warmup: the HAM clock gate holds the PE at 1.2 GHz until it has
    # seen ~3.4 us of sustained matmul activity; bridge most of the gap to
    # the first data landing (~11 us) with dummy matmuls on memset scratch.
    warm_sb = const_pool.tile([P, 2, 256], fp8, name="warm_sb")
    nc.vector.memset(warm_sb[:], 0.0)
    warm_ps = psum_pool.tile([P, 256], f32, tag="ps", name="warm_ps")
    for _ in range(8):
        nc.tensor.matmul(
            warm_ps[:], warm_sb[:, :, :128], warm_sb[:], perf_mode=DoubleRow
        )

    # Resident activations: xh.T as [128, KT, BC] fp8 (pre-swizzled in DRAM,
    # so each DMA is one contiguous line per partition). Quartered so the
    # first k-pairs land (and matmuls start) before the whole 4 MB arrives.
    # xh is alone on the Activation engine's DMA ring.
    xh_sb = xh_pool.tile([P, KT, BC], fp8, name="xh_sb")
    XQ = max(KT // 4, 1)
    for q in range(KT // XQ):
        nc.scalar.dma_start(
            xh_sb[:, q * XQ : (q + 1) * XQ, :], xh8[:, q * XQ : (q + 1) * XQ, :]
        )

    # Biases as [128, JT] (pre-swizzled on host: column j holds
    # bias[j*128:(j+1)*128]) so the DMA is one contiguous line per partition.
    bias_sb = {}
    for name, ap in (("z", bz), ("r", br), ("n", bn)):
        t = const_pool.tile([P, JT], f32, name=f"bias_{name}")
        nc.scalar.dma_start(t[:], ap)
        bias_sb[name] = t

    def load_w(w_ap, j, name):
        """[128, KT, 128] fp8 tile: [:, t, :] holds W[t*128+p, j*128+..]."""
        wt = w_pool.tile([P, KT, P], fp8, tag="w", name=name)
        nc.sync.dma_start(wt[:], w_ap[:, j : j + 1, :, :])
        return wt

    # First JB+1 r-gate weight tiles prefetch on the sync ring up front;
    # h comes later (gated below) so the startup bandwidth goes to xh+weights.
    JB = 4 if JT % 4 == 0 else 1  # leading pair-outer j-block width
    wr_pre = {j: load_w(wr, j, "wr_j") for j in range(min(JB + 1, JT))}

    # h_prev.T in bf16 for the elementwise tail (fp8 h would dominate error).
    h_sb = h_pool.tile([P, HT, BC], bf16, name="h_sb")

    # r * h_prev (transposed) in fp8, filled during the r phase.
    rh_sb = rh_pool.tile([P, HT, BC], fp8, name="rh_sb")

    def xh_pair(tp, b_i):
        return xh_sb[:, 2 * tp : 2 * tp + 2, b_i * NF : (b_i + 1) * NF]

    def accumulate(ps, w_tile, rhs_of_pair):
        for tp in range(TP):
            lhsT = w_tile[:, 2 * tp : 2 * tp + 2, :]
            for b_i in range(NB):
                nc.tensor.matmul(
                    ps[b_i][:],
                    lhsT,
                    rhs_of_pair(tp, b_i),
                    start=(tp == 0),
                    stop=(tp == TP - 1),
                    perf_mode=DoubleRow,
                )

    def r_tail(j, ps):
        r_j = act_pool.tile([P, BC], bf16, tag="r", name="r_j")
        for b_i in range(NB):
            nc.scalar.activation(
                r_j[:, b_i * NF : (b_i + 1) * NF],
                ps[b_i][:],
                Sigmoid,
                bias=bias_sb["r"][:, j : j + 1],
                scale=INV,
            )
        nc.vector.tensor_mul(rh_sb[:, j : j + 1, :], r_j[:], h_sb[:, j : j + 1, :])

    # ---- phase R: r gate, then rh = r * h_prev ----
    # The first JB tiles accumulate pair-outer in lockstep (JB*NB PSUM
    # groups): the PE consumes each freshly-DMA'd xh quarter JB x slower
    # than a single tile would, so the initial xh stream hides completely
    # under matmuls instead of stalling tile 0. (The HAM clock also warms
    # up during this stretch.)
    # Pair quarter-groups aligned with the xh quarter DMAs; within a
    # quarter go j-sequential so the first matmuls need only wr_0 + xh q0
    # and each later tile/quarter arrives just-in-time.
    pss = {j: [psum_pool.tile([P, NF], f32, tag="ps", name="ps_r")
               for _ in range(NB)] for j in range(JB)}
    TQ = max(TP // 4, 1)
    for tq in range(TP // TQ):
        for j in range(JB):
            for tp in range(tq * TQ, (tq + 1) * TQ):
                lhsT = wr_pre[j][:, 2 * tp : 2 * tp + 2, :]
                for b_i in range(NB):
                    nc.tensor.matmul(
                        pss[j][b_i][:],
                        lhsT,
                        xh_pair(tp, b_i),
                        start=(tp == 0),
                        stop=(tp == TP - 1),
                        perf_mode=DoubleRow,
                    )

    # Gate h's 4 MB DMA behind the first matmul block: a 1-element copy
    # from a block PSUM into h_sb makes the (whole-tile) h DMA a
    # write-after-write successor, so its transfer doesn't steal startup
    # bandwidth from xh/weights. The DMA then overwrites the junk element
    # with the real value. h is only consumed by rh and the combine, both
    # far later.
    nc.vector.tensor_copy(h_sb[:, 0:1, 0:1], pss[0][0][:, 0:1])
    nc.sync.dma_start(h_sb[:, : HT // 2, :], hb[:, : HT // 2, :])
    nc.sync.dma_start(h_sb[:, HT // 2 :, :], hb[:, HT // 2 :, :])

    for j in range(JB):
        r_tail(j, pss[j])

    for j in range(JB, JT):
        wr_j = wr_pre.pop(j) if j in wr_pre else load_w(wr, j, "wr_j")
        ps = [psum_pool.tile([P, NF], f32, tag="ps", name="ps_r") for _ in range(NB)]
        accumulate(ps, wr_j, xh_pair)
        r_tail(j, ps)

    # ---- phase NZ: z and n gates + combine ----
    for j in range(JT):
        wn_j = load_w(wn, j, "wn_j")
        wz_j = load_w(wz, j, "wz_j")
        psn = [psum_pool.tile([P, NF], f32, tag="ps", name="ps_n") for _ in range(NB)]
        psz = [psum_pool.tile([P, NF], f32, tag="ps", name="ps_z") for _ in range(NB)]

        def n_rhs(tp, b_i):
            if tp < XP:
                return xh_pair(tp, b_i)
            tt = tp - XP
            return rh_sb[:, 2 * tt : 2 * tt + 2, b_i * NF : (b_i + 1) * NF]

        # n-gate matmuls first: its activation + (n - h) then overlap the
        # z-gate matmuls, leaving only sigmoid -> mul -> add -> store after
        # the last matmul. Intermediates in bf16 for 2x DVE throughput.
        accumulate(psn, wn_j, n_rhs)
        n_j = act_pool.tile([P, BC], bf16, tag="n", name="n_j")
        d_j = act_pool.tile([P, BC], bf16, tag="d", name="d_j")
        for b_i in range(NB):
            sl = slice(b_i * NF, (b_i + 1) * NF)
            nc.scalar.activation(
                n_j[:, sl], psn[b_i][:], Tanh,
                bias=bias_sb["n"][:, j : j + 1], scale=INV,
            )
            nc.vector.tensor_sub(d_j[:, sl], n_j[:, sl], h_sb[:, j : j + 1, sl])

        accumulate(psz, wz_j, xh_pair)
        z_j = act_pool.tile([P, BC], bf16, tag="z", name="z_j")
        zd_j = act_pool.tile([P, BC], bf16, tag="zd", name="zd_j")
        o_j = out_pool.tile([P, BC], f32, name="o_j")
        # Last tile: quarter the post-matmul chain so the final
        # sigmoid->mul->add->store tail is 4x shorter.
        QS = NF // 2 if j == JT - 1 else NF
        for q in range(BC // QS):
            sl = slice(q * QS, (q + 1) * QS)
            h_b = h_sb[:, j : j + 1, sl]
            ps_q = psz[q * QS // NF][:, q * QS % NF : q * QS % NF + QS]
            nc.scalar.activation(
                z_j[:, sl], ps_q, Sigmoid,
                bias=bias_sb["z"][:, j : j + 1], scale=INV,
            )
            nc.vector.tensor_mul(zd_j[:, sl], z_j[:, sl], d_j[:, sl])
            nc.vector.tensor_add(o_j[:, sl], zd_j[:, sl], h_b)
            nc.sync.dma_start(out[j * P : (j + 1) * P, sl], o_j[:, sl])


_CACHED = {}


def _build(cfg=CFG, key="full", hw=True):
    if key in _CACHED:
        return _CACHED[key]
    nc = bacc.Bacc(
        "TRN2", target_bir_lowering=False, debug=False, enable_asserts=False
    )
    BC, K, H, KT, JT = cfg.BC, cfg.K, cfg.H, cfg.KT, cfg.JT
    xh8 = nc.dram_tensor("xh8", [P, KT, BC], fp8, kind="ExternalInput").ap()
    hb = nc.dram_tensor("hb", [P, cfg.HT, BC], bf16, kind="ExternalInput").ap()
    wz = nc.dram_tensor("wz", [P, JT, KT, P], fp8, kind="ExternalInput").ap()
    wr = nc.dram_tensor("wr", [P, JT, KT, P], fp8, kind="ExternalInput").ap()
    wn = nc.dram_tensor("wn", [P, JT, KT, P], fp8, kind="ExternalInput").ap()
    bz = nc.dram_tensor("bz", [P, JT], f32, kind="ExternalInput").ap()
    br = nc.dram_tensor("br", [P, JT], f32, kind="ExternalInput").ap()
    bn = nc.dram_tensor("bn", [P, JT], f32, kind="ExternalInput").ap()
    out = nc.dram_tensor("out", [H, BC], f32, kind="ExternalOutput").ap()

    with tile.TileContext(nc) as tc:
        _gru_tile_kernel(tc, cfg, xh8, hb, wz, wr, wn, bz, br, bn, out)
    nc.compile()
    if hw:
        nc.m = get_hw_module(nc.m)
    _CACHED[key] = nc
    return nc


def _swizzle_w(W, cfg):
    """[K, H] -> [P, JT, KT, P]: w[p, j, t, m] = W[t*128+p, j*128+m]."""
    w8 = (np.asarray(W, dtype=np.float32) * WSCALE).astype(np_f8)
    w8 = w8.reshape(cfg.KT, P, cfg.JT, P).transpose(1, 2, 0, 3)
    return np.ascontiguousarray(w8)


def _swizzle_act(a_t, nt, bc, dtype):
    """[nt*P, BC] -> [P, nt, BC]: out[p, t, :] = a_t[t*128+p, :]."""
    a = np.asarray(a_t).reshape(nt, P, bc).transpose(1, 0, 2)
    return np.ascontiguousarray(a.astype(dtype))


def _make_in_maps(x, h_prev, W_z, b_z, W_r, b_r, W_n, b_n, cfg=CFG):
    wz8 = _swizzle_w(W_z, cfg)
    wr8 = _swizzle_w(W_r, cfg)
    wn8 = _swizzle_w(W_n, cfg)

    def _swz_b(b):  # [H] -> [P, JT]: col j = b[j*128:(j+1)*128]
        return np.ascontiguousarray(
            np.asarray(b, dtype=np.float32).reshape(cfg.JT, P).T
        )

    bz32 = _swz_b(b_z)
    br32 = _swz_b(b_r)
    bn32 = _swz_b(b_n)
    in_maps = []
    for i in range(N_CORES):
        sl = slice(i * cfg.BC, (i + 1) * cfg.BC)
        xh_i = np.concatenate([x[sl].T, h_prev[sl].T], axis=0)
        in_maps.append(
            {
                "xh8": _swizzle_act(xh_i, cfg.KT, cfg.BC, np_f8),
                "hb": _swizzle_act(h_prev[sl].T, cfg.HT, cfg.BC, np_bf16),
                "wz": wz8,
                "wr": wr8,
                "wn": wn8,
                "bz": bz32,
                "br": br32,
                "bn": bn32,
            }
        )
    return in_maps


LAST_RESULT = None


def kernel(x, h_prev, W_z, b_z, W_r, b_r, W_n, b_n):
    global LAST_RESULT
    trace = bool(os.environ.get("GRU_TRACE"))
    if trace:
        _install_ntff_hook()
    nc = _build()
    in_maps = _make_in_maps(x, h_prev, W_z, b_z, W_r, b_r, W_n, b_n)
    res = run_bass_kernel_spmd(
        nc, in_maps, core_ids=list(range(N_CORES)), trace=trace
    )
    LAST_RESULT = res
    outs = [res.results[i]["out"].T for i in range(N_CORES)]
    return np.ascontiguousarray(np.concatenate(outs, axis=0).astype(np.float32))
